# revision 18
# baseline (speedup 1.0000x reference)
"""Trainium2 Bass kernel for nn_DecoderBlock (B=4,T=S=E=1024,H=16,D=64) on 8 cores.

Sharding: data-parallel over batch B=4 x tensor-parallel 2 (heads 16->8,
FFN hidden 4096->2048), AllReduce between core pairs after each projection.

Layout trick: the whole residual stream is kept TRANSPOSED on-chip as
x^T [E(partitions), T(free)] so every matmul's operands are already in the
natural lhsT/rhs layout and no PE transposes are needed.  LayerNorm reduces
over the partition dim via ones-matmuls; the per-token mean/rstd rows are
broadcast back across partitions with tiny K=1 / K=2 matmuls.
Softmax denominators come for free from an extra ones-column appended to V.
"""
import sys

sys.path.insert(0, "/opt/trn_rl_repo")

import numpy as np
import ml_dtypes

import concourse.bass as bass
import concourse.bacc as bacc
import concourse.mybir as mybir
import concourse.tile as tile

BF16 = mybir.dt.bfloat16
F32 = mybir.dt.float32
F8 = mybir.dt.float8e4
DR = mybir.MatmulPerfMode.DoubleRow
AF = mybir.ActivationFunctionType
OP = mybir.AluOpType

W8 = 64.0                      # fp8 weight pre-scale (keeps e4m3 in normal range)
SC8 = 1.0 / (W8 * W8)          # undo two fp8 weight pre-scales

B, T, S, E, H, D = 4, 1024, 1024, 1024, 16, 64
HL = H // 2          # heads per core (TP-2)
FF = 4 * E // 2      # ffn hidden per core
KO = E // 128        # 8 partition subtiles of E
NC_ = 512            # matmul free-dim chunk
CC = T // NC_        # 2 chunks over T
PAIRS = [[0, 1], [2, 3], [4, 5], [6, 7]]


SKIP_CC = False


def build(nbody=1):
    nc = bacc.Bacc(num_devices=8)

    def P(name, shape, dt):
        return nc.declare_dram_parameter(name, shape, dt, isOutput=False)

    xT = P("xT", [E, T], F32)
    caT = P("caT", [E, S], BF16)
    wq, wk, wv = P("wq", [E, 512], BF16), P("wk", [E, 512], BF16), P("wv", [E, 512], BF16)
    wqc, wkc, wvc = P("wqc", [E, 512], BF16), P("wkc", [E, 512], BF16), P("wvc", [E, 512], BF16)
    wo, woc = P("wo", [512, E], BF16), P("woc", [512, E], BF16)
    w1, w2 = P("w1", [E, FF], BF16), P("w2", [FF, E], BF16)
    gb = [P(f"gb{i}", [2, E], BF16) for i in (1, 2, 3)]
    gpp_d = [P(f"g{i}", [E], F32) for i in (1, 2, 3)]
    bo2, bo2c, b22 = P("bo2", [E], F32), P("bo2c", [E], F32), P("b22", [E], F32)
    b1r = P("b1r", [FF], F32)
    cmask = P("cmask", [128, 4, 512], BF16)
    out_xT = nc.declare_dram_parameter("out_xT", [E, T], F32, isOutput=True)

    with tile.TileContext(nc) as tc:
        with tc.tile_pool(name="persist", bufs=1) as pp:
            xT_sb = pp.tile([128, KO, T], F32, tag="xT")
            for ko in range(KO):
                nc.sync.dma_start(out=xT_sb[:, ko, :],
                                  in_=xT[ko * 128:(ko + 1) * 128, :])
            ca_sb = pp.tile([128, KO, S], BF16, tag="ca")
            nc.sync.dma_start(out=ca_sb[:], in_=caT.rearrange("(ko p) t -> p ko t", p=128))
            cm_sb = pp.tile([128, 4, 512], BF16, tag="cm")
            nc.sync.dma_start(out=cm_sb[:], in_=cmask[:])
            ones_bf = pp.tile([128, 512], BF16, tag="ones")
            nc.vector.memset(ones_bf[:], 1.0)
            gl_sb, bl_sb, gpp = [], [], []
            for i in range(3):
                ta = pp.tile([1, KO, 128], BF16, tag=f"gl{i}")
                nc.sync.dma_start(out=ta[:], in_=gb[i].rearrange("a (ko m) -> a ko m", m=128)[0:1])
                gl_sb.append(ta)
                tb = pp.tile([1, KO, 128], BF16, tag=f"bl{i}")
                nc.sync.dma_start(out=tb[:], in_=gb[i].rearrange("a (ko m) -> a ko m", m=128)[1:2])
                bl_sb.append(tb)
                t2 = pp.tile([128, KO], F32, tag=f"gpp{i}")
                with nc.allow_non_contiguous_dma(reason="tiny LN vector"):
                    nc.sync.dma_start(out=t2[:], in_=gpp_d[i].rearrange("(ko p) -> p ko", p=128))
                gpp.append(t2)
            bpp = []
            for nm, d in (("bo2", bo2), ("bo2c", bo2c), ("b22", b22)):
                t_ = pp.tile([128, KO], F32, tag=nm)
                with nc.allow_non_contiguous_dma(reason="tiny bias vector"):
                    nc.sync.dma_start(out=t_[:], in_=d.rearrange("(ko p) -> p ko", p=128))
                bpp.append(t_)
            eps_t = pp.tile([1, 1], F32, tag="eps")
            nc.vector.memset(eps_t[:], 1e-5)
            b1pp = pp.tile([128, FF // 128], F32, tag="b1")
            with nc.allow_non_contiguous_dma(reason="tiny bias vector"):
                nc.sync.dma_start(out=b1pp[:], in_=b1r.rearrange("(m p) -> p m", p=128))

            for ibody in range(nbody):
                _body(nc, tc, ibody, xT_sb, ca_sb, cm_sb, ones_bf, (gl_sb, bl_sb), gpp,
                      bpp, b1pp, eps_t,
                      dict(wq=wq, wk=wk, wv=wv, wqc=wqc, wkc=wkc, wvc=wvc,
                           wo=wo, woc=woc, w1=w1, w2=w2, xT=xT),
                      out_xT)
    nc.finalize()
    return nc


def _body(nc, tc, ibody, xT_sb, ca_sb, cm_sb, ones_bf, gbl, gpp,
          bpp, b1pp, eps_t, W, out_xT):
    gl_sb, bl_sb = gbl
    bo2pp, bo2cpp, b22pp = bpp
    ar = {}
    for k in (1, 2, 3):
        ar[k] = [(nc.dram_tensor(f"ar{k}_{ibody}_{c}_in", [E, NC_], F32),
                  nc.dram_tensor(f"ar{k}_{ibody}_{c}_out", [E, NC_], F32))
                 for c in range(CC)]

    if ibody > 0:
        # re-load pristine x for the timing replica
        for ko in range(KO):
            nc.sync.dma_start(out=xT_sb[:, ko, :],
                              in_=W["xT"][ko * 128:(ko + 1) * 128, :])

    with tc.tile_pool(name=f"A{ibody}", bufs=1) as pa, \
         tc.tile_pool(name=f"ps{ibody}", bufs=8, space="PSUM") as pspool:

        def ps():
            return pspool.tile([128, NC_], F32, tag="ps", name="ps")

        def layer_norm(i):
            """LN over partitions of xT_sb -> bf16 tile [128, KO, T]."""
            ln = pa.tile([128, KO, T], BF16, tag="lnout", name="ln")
            for c in range(CC):
                cs = slice(c * NC_, (c + 1) * NC_)
                xb = pa.tile([128, KO, NC_], BF16, tag="stat", bufs=2, name="xb")
                for ko in range(KO):
                    nc.scalar.copy(out=xb[:, ko, :], in_=xT_sb[:, ko, cs])
                sq = pa.tile([128, KO, NC_], BF16, tag="stat", bufs=2, name="sq")
                nc.scalar.activation(sq[:], xb[:], AF.Square)
                ps1, ps2 = ps(), ps()
                for ko in range(KO):
                    nc.tensor.matmul(ps1[0:1, :], ones_bf[:, 0:1], xb[:, ko, :],
                                     start=(ko == 0), stop=(ko == KO - 1))
                for ko in range(KO):
                    nc.tensor.matmul(ps2[0:1, :], ones_bf[:, 0:1], sq[:, ko, :],
                                     start=(ko == 0), stop=(ko == KO - 1))
                m_ = pa.tile([1, NC_], F32, tag="row_m", bufs=1, name="m_")
                nc.vector.tensor_scalar_mul(m_[:], ps1[0:1, :], 1.0 / E)
                msq = pa.tile([1, NC_], F32, tag="row_q", bufs=1, name="msq")
                nc.vector.tensor_mul(msq[:], m_[:], m_[:])
                var = pa.tile([1, NC_], F32, tag="row_v", bufs=1, name="var")
                nc.vector.scalar_tensor_tensor(var[:], ps2[0:1, :], 1.0 / E,
                                               msq[:], OP.mult, OP.subtract)
                sqv = pa.tile([1, NC_], F32, tag="row_s", bufs=1, name="sqv")
                nc.scalar.activation(sqv[:], var[:], AF.Sqrt, bias=eps_t[:])
                rstd = pa.tile([1, NC_], F32, tag="row_r", bufs=1, name="rstd")
                nc.vector.reciprocal(rstd[:], sqv[:])
                rbf = pa.tile([1, NC_], BF16, tag="rowsb2", bufs=1, name="rbf")
                nc.vector.tensor_copy(rbf[:], rstd[:])
                nmr = pa.tile([1, NC_], BF16, tag="rowsb1", bufs=1, name="nmr")
                # nmr = -m * rstd
                nc.vector.scalar_tensor_tensor(nmr[:], m_[:], -1.0,
                                               rstd[:], OP.mult, OP.mult)
                rbc = ps()
                nc.tensor.matmul(rbc[:, :], ones_bf[0:1, 0:128], rbf[:],
                                 start=True, stop=True)
                for ko in range(KO):
                    bbc = ps()
                    nc.tensor.matmul(bbc[:, :], gl_sb[i][:, ko, :], nmr[:],
                                     start=True, stop=False)
                    nc.tensor.matmul(bbc[:, :], bl_sb[i][:, ko, :],
                                     ones_bf[0:1, 0:NC_], start=False, stop=True)
                    t0 = pa.tile([128, NC_], F32, tag="tmp", bufs=2, name="t0")
                    nc.vector.scalar_tensor_tensor(t0[:], xT_sb[:, ko, cs],
                                                   gpp[i][:, ko:ko + 1], rbc[:, :],
                                                   OP.mult, OP.mult)
                    nc.vector.tensor_tensor(ln[:, ko, cs], t0[:], bbc[:, :], OP.add)
            return ln

        def project_qk(pb_, lnsrc, w_d, tag, bufs=1):
            """-> [128, 4, T] bf16 : rows = 2 heads x 64, per pair j."""
            w_sb = pb_.tile([128, KO, 512], BF16, tag="wqkv", bufs=2, name="wsb")
            nc.sync.dma_start(out=w_sb[:], in_=w_d.rearrange("(ko p) m -> p ko m", p=128))
            qt = pb_.tile([128, 4, T], BF16, tag=tag, bufs=bufs, name="qt")
            for j in range(4):
                for c in range(CC):
                    p_ = ps()
                    for ko in range(KO):
                        nc.tensor.matmul(p_[:, :], w_sb[:, ko, j * 128:(j + 1) * 128],
                                         lnsrc[:, ko, c * NC_:(c + 1) * NC_],
                                         start=(ko == 0), stop=(ko == KO - 1))
                    nc.vector.tensor_copy(qt[:, j, c * NC_:(c + 1) * NC_], p_[:, :])
            return qt

        def project_v(pb_, src, w_d):
            """-> [128, 8, 8, 65] bf16 : [s_part, s_sub, head, d|ones]."""
            w_sb = pb_.tile([128, KO, 512], BF16, tag="wqkv", bufs=2, name="wsb")
            nc.sync.dma_start(out=w_sb[:], in_=w_d.rearrange("(ko p) m -> p ko m", p=128))
            vv = pb_.tile([128, 8, HL, 65], BF16, tag="vv", name="vv")
            for s in range(8):
                p_ = ps()
                for ko in range(KO):
                    nc.tensor.matmul(p_[:, :], src[:, ko, s * 128:(s + 1) * 128],
                                     w_sb[:, ko, :], start=(ko == 0), stop=(ko == KO - 1))
                nc.scalar.copy(out=vv[:, s, :, 0:64],
                               in_=p_[:, :].rearrange("p (h d) -> p h d", d=64))
                nc.vector.memset(vv[:, s, :, 64:65], 1.0)
            return vv

        def attention(pb_, qt, kt, vv, causal):
            onorm = pb_.tile([128, 4, T], BF16, tag="onorm", name="onorm")
            for c in range(CC):
                for h in range(HL):
                    j, half = h // 2, h % 2
                    pb = 64 * half
                    subs = list(range(4 * (c + 1))) if causal else list(range(8))
                    eb = pb_.tile([128, 8, NC_], BF16, tag="expb", bufs=2, name="eb")
                    for s_ in subs:
                        p_ = ps()
                        nc.tensor.matmul(p_[:, :],
                                         kt[pb:pb + 64, j, s_ * 128:(s_ + 1) * 128],
                                         qt[pb:pb + 64, j, c * NC_:(c + 1) * NC_],
                                         start=True, stop=True)
                        nc.scalar.activation(eb[:, s_, :], p_[:, :], AF.Exp)
                        if causal and s_ >= 4 * c:
                            nc.vector.tensor_mul(eb[:, s_, :], eb[:, s_, :],
                                                 cm_sb[:, s_ - 4 * c, :])
                    av = ps()
                    for i_, s_ in enumerate(subs):
                        nc.tensor.matmul(av[0:65, :], vv[:, s_, h, :], eb[:, s_, :],
                                         start=(i_ == 0), stop=(i_ == len(subs) - 1))
                    rr = pb_.tile([65, NC_], F32, tag="row_rr", bufs=2, name="rr")
                    nc.vector.reciprocal(rr[64:65, :], av[64:65, :])
                    rb = pb_.tile([65, NC_], BF16, tag="row_rrb", bufs=2, name="rb")
                    nc.vector.tensor_copy(rb[64:65, :], rr[64:65, :])
                    bc = ps()
                    nc.tensor.matmul(bc[0:64, :], ones_bf[64:65, 0:64], rb[64:65, :],
                                     start=True, stop=True)
                    bcs = pb_.tile([64, NC_], BF16, tag="bcs", bufs=2, name="bcs")
                    nc.vector.tensor_copy(bcs[:, :], bc[0:64, :])
                    nc.vector.tensor_tensor(onorm[pb:pb + 64, j, c * NC_:(c + 1) * NC_],
                                            av[0:64, :], bcs[:, :], OP.mult)
            return onorm

        def out_proj(pb_, onorm, wo_d, ark, bias_pp):
            wo_sb = pb_.tile([128, 4, E], BF16, tag="wo", name="wo_sb")
            nc.sync.dma_start(out=wo_sb[:], in_=wo_d.rearrange("(ks p) e -> p ks e", p=128))
            for c in range(CC):
                for m in range(KO):
                    p_ = ps()
                    for ks in range(4):
                        nc.tensor.matmul(p_[:, :], wo_sb[:, ks, m * 128:(m + 1) * 128],
                                         onorm[:, ks, c * NC_:(c + 1) * NC_],
                                         start=(ks == 0), stop=(ks == 3))
                    st = pa.tile([128, NC_], F32, tag="arst", bufs=4, name="st")
                    nc.vector.tensor_scalar_add(st[:, :], p_[:, :], bias_pp[:, m:m + 1])
                    nc.sync.dma_start(out=ark[c][0][m * 128:(m + 1) * 128, :], in_=st[:])
                allreduce_c(ark, c)

        def allreduce_c(ark, c):
            a_in, a_out = ark[c]
            if SKIP_CC:
                nc.sync.dma_start(out=a_out[:], in_=a_in[:])
            else:
                nc.gpsimd.collective_compute(
                    "AllReduce", OP.add, replica_groups=PAIRS,
                    ins=[a_in[:]], outs=[a_out[:]])
            nc.gpsimd.dma_start(
                out=xT_sb[:, :, c * NC_:(c + 1) * NC_],
                in_=a_out.rearrange("(ko p) t -> p ko t", p=128),
                accum_op=OP.add)



        with tc.tile_pool(name=f"B{ibody}", bufs=1) as pb_:
            # ---- self attention ----
            ln1 = layer_norm(0)
            qt = project_qk(pb_, ln1, W["wq"], "qt")
            kt = project_qk(pb_, ln1, W["wk"], "kt")
            vv = project_v(pb_, ln1, W["wv"])
            on1 = attention(pb_, qt, kt, vv, causal=True)
            out_proj(pb_, on1, W["wo"], ar[1], bo2pp)
            # cross K/V from raw ca — independent of AR1, fills the gap
            ktc = project_qk(pb_, ca_sb, W["wkc"], "kt")
            vvc = project_v(pb_, ca_sb, W["wvc"])

            # ---- cross attention ----
            ln2 = layer_norm(1)
            qtc = project_qk(pb_, ln2, W["wqc"], "qt")
            on2 = attention(pb_, qtc, ktc, vvc, causal=False)
            out_proj(pb_, on2, W["woc"], ar[2], bo2cpp)

        # ---- FFN ----
        ln3 = layer_norm(2)
        with tc.tile_pool(name=f"C{ibody}", bufs=1) as pc:
            ht = pc.tile([128, FF // 128, T], BF16, tag="ht", name="ht")
            for m in range(FF // 128):
                w1m = pc.tile([128, KO, 128], BF16, tag="w1m", bufs=4, name="w1m")
                nc.sync.dma_start(
                    out=w1m[:],
                    in_=W["w1"].rearrange("(ko p) f -> p ko f", p=128)[:, :, m * 128:(m + 1) * 128])
                for c in range(CC):
                    p_ = ps()
                    for ko in range(KO):
                        nc.tensor.matmul(p_[:, :], w1m[:, ko, :],
                                         ln3[:, ko, c * NC_:(c + 1) * NC_],
                                         start=(ko == 0), stop=(ko == KO - 1))
                    nc.scalar.activation(ht[:, m, c * NC_:(c + 1) * NC_], p_[:, :],
                                         AF.Relu, bias=b1pp[:, m:m + 1])
            w2m_t = [None] * KO
            for m in range(KO):
                w2m = pc.tile([128, FF // 128, 128], BF16, tag="w2m", bufs=8, name="w2m")
                nc.sync.dma_start(
                    out=w2m[:],
                    in_=W["w2"].rearrange("(ks p) e -> p ks e", p=128)[:, :, m * 128:(m + 1) * 128])
                w2m_t[m] = w2m
            for c in range(CC):
                for m in range(KO):
                    p_ = ps()
                    for ks in range(FF // 128):
                        nc.tensor.matmul(p_[:, :], w2m_t[m][:, ks, :],
                                         ht[:, ks, c * NC_:(c + 1) * NC_],
                                         start=(ks == 0), stop=(ks == FF // 128 - 1))
                    st = pa.tile([128, NC_], F32, tag="arst", bufs=4, name="st")
                    nc.vector.tensor_scalar_add(st[:, :], p_[:, :], b22pp[:, m:m + 1])
                    nc.sync.dma_start(out=ar[3][c][0][m * 128:(m + 1) * 128, :], in_=st[:])
                allreduce_c(ar[3], c)
                for ko in range(KO):
                    nc.sync.dma_start(
                        out=out_xT[ko * 128:(ko + 1) * 128, c * NC_:(c + 1) * NC_],
                        in_=xT_sb[:, ko, c * NC_:(c + 1) * NC_])


# ------------------------------------------------------------------ host side

_CACHE = {}


COMM_FREE = True


def _get_runner(nbody=1):
    key = (nbody, COMM_FREE)
    if key in _CACHE:
        return _CACHE[key]
    import jax
    from jax.sharding import Mesh, PartitionSpec
    from jax.experimental.shard_map import shard_map
    from concourse.bass2jax import (_bass_exec_p, install_neuronx_cc_hook,
                                    partition_id_tensor)

    nc = (build_nocc if COMM_FREE else build)(nbody)
    install_neuronx_cc_hook()
    pn = nc.partition_id_tensor.name if nc.partition_id_tensor else None
    in_names, out_names, out_avals = [], [], []
    for alloc in nc.m.functions[0].allocations:
        if not isinstance(alloc, mybir.MemoryLocationSet):
            continue
        name = alloc.memorylocations[0].name
        if alloc.kind == "ExternalInput":
            if name != pn:
                in_names.append(name)
        elif alloc.kind == "ExternalOutput":
            out_names.append(name)
            out_avals.append(jax.core.ShapedArray(
                tuple(alloc.tensor_shape), mybir.dt.np(alloc.dtype)))
    n_params = len(in_names)
    all_in = in_names + out_names + ([pn] if pn else [])

    def _jbody(*args):
        ops = list(args)
        if pn:
            ops.append(partition_id_tensor())
        return tuple(_bass_exec_p.bind(
            *ops, out_avals=tuple(out_avals), in_names=tuple(all_in),
            out_names=tuple(out_names), lowering_input_output_aliases=(),
            sim_require_finite=True, sim_require_nnan=True, nc=nc))

    devices = jax.devices()[:8]
    mesh = Mesh(np.asarray(devices), ("core",))
    spec = (PartitionSpec("core"),)
    fn = jax.jit(shard_map(_jbody, mesh=mesh,
                           in_specs=spec * (n_params + len(out_names)),
                           out_specs=spec * len(out_names), check_rep=False),
                 keep_unused=True)
    _CACHE[key] = (fn, in_names, out_names, out_avals)
    return _CACHE[key]


def _make_core_inputs(c, inp):
    bf = ml_dtypes.bfloat16
    b, r = divmod(c, 2)
    hs = slice(8 * r, 8 * r + 8)
    sc = float(E) ** -0.5

    def stack_heads(w):  # [8, E, D] -> [E, 512]
        return np.ascontiguousarray(np.transpose(w, (1, 0, 2)).reshape(E, 512))

    p, f = np.arange(128)[:, None, None], np.arange(512)[None, None, :]
    jj = np.arange(4)[None, :, None]
    cmask = (f >= 128 * jj + p).astype(bf)

    return {
        "xT": np.ascontiguousarray(inp["x"][b].T).astype(np.float32),
        "caT": np.ascontiguousarray(inp["ca"][b].T).astype(bf),
        "wq": (stack_heads(inp["Wq_s"][hs]) * sc).astype(bf),
        "wk": stack_heads(inp["Wk_s"][hs]).astype(bf),
        "wv": stack_heads(inp["Wv_s"][hs]).astype(bf),
        "wqc": (stack_heads(inp["Wq_c"][hs]) * sc).astype(bf),
        "wkc": stack_heads(inp["Wk_c"][hs]).astype(bf),
        "wvc": stack_heads(inp["Wv_c"][hs]).astype(bf),
        "wo": np.ascontiguousarray(inp["Wo_s"][512 * r:512 * (r + 1), :]).astype(bf),
        "woc": np.ascontiguousarray(inp["Wo_c"][512 * r:512 * (r + 1), :]).astype(bf),
        "w1": np.ascontiguousarray(inp["W1"][:, FF * r:FF * (r + 1)]).astype(bf),
        "w2": np.ascontiguousarray(inp["W2"][FF * r:FF * (r + 1), :]).astype(bf),
        "gb1": np.stack([inp["ln1_g"], inp["ln1_b"]]).astype(bf),
        "gb2": np.stack([inp["ln2_g"], inp["ln2_b"]]).astype(bf),
        "gb3": np.stack([inp["ln3_g"], inp["ln3_b"]]).astype(bf),
        "g1": np.asarray(inp["ln1_g"], np.float32),
        "g2": np.asarray(inp["ln2_g"], np.float32),
        "g3": np.asarray(inp["ln3_g"], np.float32),
        "bo2": np.asarray(inp["bo_s"], np.float32) * 0.5,
        "bo2c": np.asarray(inp["bo_c"], np.float32) * 0.5,
        "b22": np.asarray(inp["b2"], np.float32) * 0.5,
        "b1r": np.asarray(inp["b1"][FF * r:FF * (r + 1)], np.float32),
        "cmask": cmask,
    }


def _run(nbody, in_maps, dev_inputs=None, dev_zeros=None, download=True):
    import jax
    fn, in_names, out_names, out_avals = _get_runner(nbody)
    if dev_inputs is None:
        concat = [np.concatenate([np.asarray(in_maps[c][n]) for c in range(8)], axis=0)
                  for n in in_names]
        dev_inputs = [jax.device_put(a) for a in concat]
    if dev_zeros is None:
        dev_zeros = [jax.device_put(np.zeros((8 * a.shape[0], *a.shape[1:]), a.dtype))
                     for a in out_avals]
    outs = fn(*dev_inputs, *dev_zeros)
    for o in outs:
        o.block_until_ready()
    if not download:
        return None, (dev_inputs, dev_zeros)
    res = []
    for c in range(8):
        res.append({n: np.asarray(outs[i]).reshape(8, *out_avals[i].shape)[c]
                    for i, n in enumerate(out_names)})
    return res, (dev_inputs, dev_zeros)


def kernel(**inputs):
    inp = {k: np.asarray(v) for k, v in inputs.items()}
    mk = _make_core_inputs_nocc if COMM_FREE else _make_core_inputs
    in_maps = [mk(c, inp) for c in range(8)]
    res, _ = _run(1, in_maps)
    if COMM_FREE:
        out = np.stack([
            np.concatenate([res[2 * b]["out_xT"], res[2 * b + 1]["out_xT"]],
                           axis=1).T
            for b in range(B)]).astype(np.float32)
    else:
        out = np.stack([res[2 * b]["out_xT"].T for b in range(B)]).astype(np.float32)
    return out


# ---------------------------------------------------------------- comm-free

def build_nocc(nbody=1):
    """Communication-free sharding: core = (batch b, T-half h).  Each core
    computes its 512 query tokens for ALL 16 heads and the full FFN, with
    K/V duplicated across the pair.  Self-attn keys are permuted so the own
    half always sits at key positions 0..511 (the per-core causal mask input
    encodes the permutation) — keeps the SPMD program identical on all cores.
    """
    nc = bacc.Bacc(num_devices=8)

    def P(name, shape, dt):
        return nc.declare_dram_parameter(name, shape, dt, isOutput=False)

    xTb = P("xTb", [E, T], BF16)        # permuted x^T, bf16 (LN1 / self K,V)
    xTo = P("xTo", [E, NC_], F32)       # own-half x^T, f32 (residual base)
    caT = P("caT", [E, S], BF16)
    wq, wk, wv = P("wq", [E, E], BF16), P("wk", [E, E], BF16), P("wv", [E, E], BF16)
    wqc, wkc, wvc = P("wqc", [E, E], BF16), P("wkc", [E, E], BF16), P("wvc", [E, E], BF16)
    wo, woc = P("wo", [E, E], BF16), P("woc", [E, E], BF16)
    w1, w2 = P("w1", [E, 4 * E], BF16), P("w2", [4 * E, E], BF16)
    gpp_d = [P(f"g{i}", [E], F32) for i in (1, 2, 3)]
    blp_d = [P(f"bl{i}", [E], F32) for i in (1, 2, 3)]
    bo_, boc_, b2_ = P("bo", [E], F32), P("boc", [E], F32), P("b2", [E], F32)
    b1r = P("b1r", [4 * E], F32)
    smask = P("smask", [128, 4, NC_], BF16)
    oflag = P("oflag", [128, 1], F32)
    out_xT = nc.declare_dram_parameter("out_xT", [E, NC_], F32, isOutput=True)

    with tile.TileContext(nc) as tc:
        with tc.tile_pool(name="persist", bufs=1) as pp:
            xTb_sb = pp.tile([128, KO, T], BF16, tag="xTb")
            for ko in range(KO):
                nc.sync.dma_start(out=xTb_sb[:, ko, :], in_=xTb[ko * 128:(ko + 1) * 128, :])
            xTo_sb = pp.tile([128, KO, NC_], F32, tag="xTo")
            nc.sync.dma_start(out=xTo_sb[:], in_=xTo.rearrange("(ko p) t -> p ko t", p=128))
            ca_sb = pp.tile([128, KO, S], BF16, tag="ca")
            nc.sync.dma_start(out=ca_sb[:], in_=caT.rearrange("(ko p) t -> p ko t", p=128))
            sm_sb = pp.tile([128, 4, NC_], BF16, tag="sm")
            nc.sync.dma_start(out=sm_sb[:], in_=smask[:])
            ones_bf = pp.tile([128, 512], BF16, tag="ones")
            nc.vector.memset(ones_bf[:], 1.0)
            fl_sb = pp.tile([128, 1], F32, tag="oflag")
            with nc.allow_non_contiguous_dma(reason="tiny flag vector"):
                nc.sync.dma_start(out=fl_sb[:], in_=oflag[:])
            gpp, blpp = [], []
            for i in range(3):
                t2 = pp.tile([128, KO], F32, tag=f"gpp{i}")
                with nc.allow_non_contiguous_dma(reason="tiny LN vector"):
                    nc.sync.dma_start(out=t2[:], in_=gpp_d[i].rearrange("(ko p) -> p ko", p=128))
                gpp.append(t2)
                t3 = pp.tile([128, KO], F32, tag=f"blpp{i}")
                with nc.allow_non_contiguous_dma(reason="tiny LN vector"):
                    nc.sync.dma_start(out=t3[:], in_=blp_d[i].rearrange("(ko p) -> p ko", p=128))
                blpp.append(t3)
            bpp = []
            for nm, d in (("bo", bo_), ("boc", boc_), ("b2", b2_)):
                t_ = pp.tile([128, KO], F32, tag=nm)
                with nc.allow_non_contiguous_dma(reason="tiny bias vector"):
                    nc.sync.dma_start(out=t_[:], in_=d.rearrange("(ko p) -> p ko", p=128))
                bpp.append(t_)
            eps_t = pp.tile([1, 1], F32, tag="eps")
            nc.vector.memset(eps_t[:], 1e-5)
            b1pp = pp.tile([128, 4 * E // 128], F32, tag="b1")
            with nc.allow_non_contiguous_dma(reason="tiny bias vector"):
                nc.sync.dma_start(out=b1pp[:], in_=b1r.rearrange("(m p) -> p m", p=128))

            for ibody in range(nbody):
                _body_nocc(nc, tc, ibody, xTb_sb, xTo_sb, ca_sb, sm_sb, ones_bf,
                           fl_sb, (gpp, blpp), bpp, b1pp, eps_t,
                           dict(wq=wq, wk=wk, wv=wv, wqc=wqc, wkc=wkc, wvc=wvc,
                                wo=wo, woc=woc, w1=w1, w2=w2, xTo=xTo),
                           out_xT)
    nc.finalize()
    return nc


def _body_nocc(nc, tc, ibody, xTb_sb, xTo_sb, ca_sb, sm_sb, ones_bf, fl_sb, gbl,
               bpp, b1pp, eps_t, W, out_xT):
    gpp, blpp = gbl
    bopp, bocpp, b2pp = bpp

    if ibody > 0:
        nc.sync.dma_start(out=xTo_sb[:],
                          in_=W["xTo"].rearrange("(ko p) t -> p ko t", p=128))

    with tc.tile_pool(name=f"A{ibody}", bufs=1) as pa, \
         tc.tile_pool(name=f"ps{ibody}", bufs=8, space="PSUM") as pspool:

        pb2_ref = [None]

        def ps():
            return pspool.tile([128, NC_], F32, tag="ps", bufs=4, name="ps")

        def psw():
            return pspool.tile([128, 2 * NC_], F32, tag="pw", bufs=2, name="pw")

        def ln_rows(i, ps1, ps2, cs_out, ln, src, src_is_bf, gsl, ncols):
            m_ = pa.tile([1, NC_], F32, tag="row_m", bufs=1, name="m_")
            nc.vector.tensor_scalar_mul(m_[:, :ncols], ps1[0:1, :ncols], 1.0 / E)
            msq = pa.tile([1, NC_], F32, tag="row_q", bufs=1, name="msq")
            nc.vector.tensor_mul(msq[:, :ncols], m_[:, :ncols], m_[:, :ncols])
            var = pa.tile([1, NC_], F32, tag="row_v", bufs=1, name="var")
            nc.vector.scalar_tensor_tensor(var[:, :ncols], ps2[0:1, :ncols], 1.0 / E,
                                           msq[:, :ncols], OP.mult, OP.subtract)
            sqv = pa.tile([1, NC_], F32, tag="row_s", bufs=1, name="sqv")
            nc.scalar.activation(sqv[:, :ncols], var[:, :ncols], AF.Sqrt, bias=eps_t[:])
            rbf = pa.tile([1, NC_], BF16, tag="rowsb2", bufs=1, name="rbf")
            with nc.allow_low_precision(reason="rstd rounds to bf16 anyway"):
                nc.vector.reciprocal(rbf[:, :ncols], sqv[:, :ncols])
            nmr = pa.tile([1, NC_], BF16, tag="rowsb1", bufs=1, name="nmr")
            nc.vector.scalar_tensor_tensor(nmr[:, :ncols], m_[:, :ncols], -1.0,
                                           rbf[:, :ncols], OP.mult, OP.mult)
            rbc = ps()
            nc.tensor.matmul(rbc[:, :ncols], ones_bf[0:1, 0:128], rbf[:, :ncols],
                             start=True, stop=True)
            nmb = ps()
            nc.tensor.matmul(nmb[:, :ncols], ones_bf[0:1, 0:128], nmr[:, :ncols],
                             start=True, stop=True)
            for ko in range(KO):
                bbc = pa.tile([128, NC_], BF16, tag="bbc", bufs=2, name="bbc")
                nc.scalar.activation(bbc[:, :ncols], nmb[:, :ncols], AF.Identity,
                                     bias=blpp[i][:, ko:ko + 1],
                                     scale=gpp[i][:, ko:ko + 1])
                t0 = pa.tile([128, NC_], F32, tag="tmp", bufs=2, name="t0")
                nc.vector.scalar_tensor_tensor(t0[:, :ncols], src[ko],
                                               gpp[i][:, ko:ko + 1], rbc[:, :ncols],
                                               OP.mult, OP.mult)
                nc.vector.tensor_tensor(ln[:, ko, cs_out], t0[:, :ncols],
                                        bbc[:, :ncols], OP.add)

        def layer_norm1():
            """full-T LN over xTb (bf16 source)."""
            ln = pa.tile([128, KO, T], BF16, tag="lnf", name="lnf")
            for c in range(CC):
                cs = slice(c * NC_, (c + 1) * NC_)
                sq = pa.tile([128, KO, NC_], BF16, tag="stat", bufs=2, name="sq")
                nc.scalar.activation(sq[:], xTb_sb[:, :, cs], AF.Square)
                ps1, ps2 = ps(), ps()
                for ko in range(KO):
                    nc.tensor.matmul(ps1[0:1, :], ones_bf[:, 0:1], xTb_sb[:, ko, cs],
                                     start=(ko == 0), stop=(ko == KO - 1))
                for ko in range(KO):
                    nc.tensor.matmul(ps2[0:1, :], ones_bf[:, 0:1], sq[:, ko, :],
                                     start=(ko == 0), stop=(ko == KO - 1))
                ln_rows(0, ps1, ps2, cs, ln,
                        [xTb_sb[:, ko, cs] for ko in range(KO)], True, None, NC_)
            return ln

        def ln_stats_step(st, ko):
            """accumulate own-half LN stats for one ko sub-tile of xTo."""
            xb, sq, ps1, ps2 = st
            nc.scalar.copy(out=xb[:, ko, :], in_=xTo_sb[:, ko, :])
            nc.scalar.activation(sq[:, ko, :], xb[:, ko, :], AF.Square)
            nc.tensor.matmul(ps1[0:1, :], ones_bf[:, 0:1], xb[:, ko, :],
                             start=(ko == 0), stop=(ko == KO - 1))
            nc.tensor.matmul(ps2[0:1, :], ones_bf[:, 0:1], sq[:, ko, :],
                             start=(ko == 0), stop=(ko == KO - 1))

        def layer_norm_h_rows(i, st):
            """own-half LN rows + normalize, after ln_stats_steps are done."""
            ln = pa.tile([128, KO, NC_], BF16, tag="lnh", bufs=1, name="lnh")
            ln_rows(i, st[2], st[3], slice(0, NC_), ln,
                    [xTo_sb[:, ko, :] for ko in range(KO)], False, None, NC_)
            return ln

        def wj_tile(w_d, j):
            """stream one 128-col slice of a [E, E] weight into SBUF."""
            w_sb = pb2_ref[0].tile([128, KO, 128], BF16, tag="wj", bufs=4, name="wj")
            nc.sync.dma_start(
                out=w_sb[:],
                in_=w_d.rearrange("(ko p) m -> p ko m", p=128)[:, :, j * 128:(j + 1) * 128])
            return w_sb

        def project_qt(lnsrc, w_d, cols):
            """Q^T for 16 heads over `cols` own tokens -> [128, 8, 512]."""
            qt = pb2_ref[0].tile([128, 8, NC_], BF16, tag="qon", bufs=2, name="qt")
            for j in range(8):
                w_sb = wj_tile(w_d, j)
                p_ = ps()
                for ko in range(KO):
                    nc.tensor.matmul(p_[:, :], w_sb[:, ko, :],
                                     lnsrc[ko], start=(ko == 0), stop=(ko == KO - 1))
                nc.vector.tensor_copy(qt[:, j, :], p_[:, :])
            return qt

        def project_kt(src, w_d):
            """K^T for 16 heads over full S -> [128, 8, 1024]."""
            kt = pb2_ref[0].tile([128, 8, T], BF16, tag="kt", name="kt")
            for j in range(8):
                w_sb = wj_tile(w_d, j)
                for c in range(CC):
                    p_ = ps()
                    for ko in range(KO):
                        nc.tensor.matmul(p_[:, :], w_sb[:, ko, :],
                                         src[:, ko, c * NC_:(c + 1) * NC_],
                                         start=(ko == 0), stop=(ko == KO - 1))
                    nc.vector.tensor_copy(kt[:, j, c * NC_:(c + 1) * NC_], p_[:, :])
            return kt

        def project_v(src, w_d, flag_other=False):
            """V for 16 heads -> [128, 8, 16, 65].  When flag_other is set,
            key blocks 4..7 (the pair's other T-half) are scaled by the
            per-core visibility flag (0 for the early-half core, 1 for the
            late-half core) so self-attention needs no mask there."""
            vv = pb2_ref[0].tile([128, 8, H, 65], BF16, tag="vv", name="vv")
            for jh in range(2):
                w_sb = pb2_ref[0].tile([128, KO, 512], BF16, tag="wvh", bufs=2, name="wsb")
                nc.sync.dma_start(
                    out=w_sb[:],
                    in_=w_d.rearrange("(ko p) m -> p ko m", p=128)[:, :, jh * 512:(jh + 1) * 512])
                for s in range(8):
                    p_ = ps()
                    for ko in range(KO):
                        nc.tensor.matmul(p_[:, :], src[:, ko, s * 128:(s + 1) * 128],
                                         w_sb[:, ko, :], start=(ko == 0), stop=(ko == KO - 1))
                    if flag_other and s >= 4:
                        nc.scalar.activation(
                            vv[:, s, jh * 8:(jh + 1) * 8, 0:64],
                            p_[:, :].rearrange("p (h d) -> p h d", d=64),
                            AF.Copy, scale=fl_sb[:, 0:1])
                        nc.scalar.activation(
                            vv[:, s, jh * 8:(jh + 1) * 8, 64:65].rearrange("p h o -> p (h o)"),
                            ones_bf[:, 0:8], AF.Copy, scale=fl_sb[:, 0:1])
                    else:
                        nc.scalar.copy(out=vv[:, s, jh * 8:(jh + 1) * 8, 0:64],
                                       in_=p_[:, :].rearrange("p (h d) -> p h d", d=64))
                        nc.vector.memset(vv[:, s, jh * 8:(jh + 1) * 8, 64:65], 1.0)
            return vv

        def attention(qt, kt, vv, onorm, masked):
            for h_ in range(H):
                j, half = h_ // 2, h_ % 2
                pb = 64 * half
                eb = pb2_ref[0].tile([128, 8, NC_], BF16, tag="expb", bufs=2, name="eb")
                for s2 in range(4):          # pairs of key blocks share one
                    pw = psw()               # 2-bank PSUM tile + one wide exp
                    for q_ in range(2):
                        s_ = 2 * s2 + q_
                        nc.tensor.matmul(pw[:, q_ * NC_:(q_ + 1) * NC_],
                                         kt[pb:pb + 64, j, s_ * 128:(s_ + 1) * 128],
                                         qt[pb:pb + 64, j, :], start=True, stop=True)
                    nc.scalar.activation(
                        eb[:, 2 * s2:2 * s2 + 2, :].rearrange("p a b -> p (a b)"),
                        pw[:, :], AF.Exp)
                    if masked and s2 < 2:
                        nc.vector.tensor_mul(
                            eb[:, 2 * s2:2 * s2 + 2, :].rearrange("p a b -> p (a b)"),
                            eb[:, 2 * s2:2 * s2 + 2, :].rearrange("p a b -> p (a b)"),
                            sm_sb[:, 2 * s2:2 * s2 + 2, :].rearrange("p a b -> p (a b)"))
                av = ps()
                for s_ in range(8):
                    nc.tensor.matmul(av[0:65, :], vv[:, s_, h_, :], eb[:, s_, :],
                                     start=(s_ == 0), stop=(s_ == 7))
                rb = pb2_ref[0].tile([65, NC_], BF16, tag="row_rrb", bufs=2, name="rb")
                with nc.allow_low_precision(reason="1/denominator rounds to bf16 anyway"):
                    nc.vector.reciprocal(rb[64:65, :], av[64:65, :])
                bc = ps()
                nc.tensor.matmul(bc[0:64, :], ones_bf[64:65, 0:64], rb[64:65, :],
                                 start=True, stop=True)
                bcs = pb2_ref[0].tile([64, NC_], BF16, tag="bcs", bufs=2, name="bcs")
                nc.vector.tensor_copy(bcs[:, :], bc[0:64, :])
                nc.vector.tensor_tensor(onorm[pb:pb + 64, j, :],
                                        av[0:64, :], bcs[:, :], OP.mult)

        def out_proj(onorm, wo_d, bias_pp, ln_stats=None):
            """project + residual-add; optionally interleave next-LN stats so
            they overlap the projection instead of stalling PE afterwards."""
            for m in range(KO):
                wom = pb2_ref[0].tile([128, KO, 128], BF16, tag="wom", bufs=2, name="wom")
                nc.sync.dma_start(
                    out=wom[:],
                    in_=wo_d.rearrange("(ks p) e -> p ks e", p=128)[:, :, m * 128:(m + 1) * 128])
                p_ = ps()
                for ks in range(KO):
                    nc.tensor.matmul(p_[:, :], wom[:, ks, :], onorm[:, ks, :],
                                     start=(ks == 0), stop=(ks == KO - 1))
                nc.vector.scalar_tensor_tensor(xTo_sb[:, m, :], p_[:, :],
                                               bias_pp[:, m:m + 1], xTo_sb[:, m, :],
                                               OP.add, OP.add)
                if ln_stats is not None:
                    ln_stats_step(ln_stats, m)

        def ln_stats_tiles():
            xb = pa.tile([128, KO, NC_], BF16, tag="stat", bufs=2, name="xb")
            sq = pa.tile([128, KO, NC_], BF16, tag="stat", bufs=2, name="sq")
            return (xb, sq, ps(), ps())

        with tc.tile_pool(name=f"B{ibody}", bufs=1) as _pb2:
            pb2_ref[0] = _pb2
            # ---- self attention ----
            ln1 = layer_norm1()
            qt = project_qt([ln1[:, ko, 0:NC_] for ko in range(KO)], W["wq"], NC_)
            kt = project_kt(ln1, W["wk"])
            vv = project_v(ln1, W["wv"], flag_other=True)
            on1 = _pb2.tile([128, 8, NC_], BF16, tag="qon", bufs=2, name="on1")
            attention(qt, kt, vv, on1, masked=True)
            st2 = ln_stats_tiles()
            out_proj(on1, W["wo"], bopp, ln_stats=st2)

            # ---- cross attention ----
            # cross K from raw ca is independent of LN2 - it keeps PE busy
            # while the serial LN2 row chain runs on DVE/ACT
            ktc = project_kt(ca_sb, W["wkc"])
            ln2 = layer_norm_h_rows(1, st2)
            qtc = project_qt([ln2[:, ko, :] for ko in range(KO)], W["wqc"], NC_)
            vvc = project_v(ca_sb, W["wvc"])
            on2 = _pb2.tile([128, 8, NC_], BF16, tag="qon", bufs=2, name="on2")
            attention(qtc, ktc, vvc, on2, masked=False)
            st3 = ln_stats_tiles()
            out_proj(on2, W["woc"], bocpp, ln_stats=st3)

        # ---- FFN ----
        ln3 = layer_norm_h_rows(2, st3)
        with tc.tile_pool(name=f"C{ibody}", bufs=1) as pc:
            FH = 4 * E // 128
            ht = pc.tile([128, FH, NC_], BF16, tag="ht", name="ht")
            for m in range(FH):
                w1m = pc.tile([128, KO, 128], BF16, tag="w1m", bufs=4, name="w1m")
                nc.sync.dma_start(
                    out=w1m[:],
                    in_=W["w1"].rearrange("(ko p) f -> p ko f", p=128)[:, :, m * 128:(m + 1) * 128])
                p_ = ps()
                for ko in range(KO):
                    nc.tensor.matmul(p_[:, :], w1m[:, ko, :], ln3[:, ko, :],
                                     start=(ko == 0), stop=(ko == KO - 1))
                nc.scalar.activation(ht[:, m, :], p_[:, :], AF.Relu,
                                     bias=b1pp[:, m:m + 1])
            for m in range(KO):
                w2m = pc.tile([128, FH, 128], BF16, tag="w2m", bufs=2, name="w2m")
                nc.sync.dma_start(
                    out=w2m[:],
                    in_=W["w2"].rearrange("(ks p) e -> p ks e", p=128)[:, :, m * 128:(m + 1) * 128])
                p_ = ps()
                for ks in range(FH):
                    nc.tensor.matmul(p_[:, :], w2m[:, ks, :], ht[:, ks, :],
                                     start=(ks == 0), stop=(ks == FH - 1))
                nc.vector.scalar_tensor_tensor(xTo_sb[:, m, :], p_[:, :],
                                               b2pp[:, m:m + 1], xTo_sb[:, m, :],
                                               OP.add, OP.add)
        for ko in range(KO):
            nc.sync.dma_start(out=out_xT[ko * 128:(ko + 1) * 128, :],
                              in_=xTo_sb[:, ko, :])


def _make_core_inputs_nocc(c, inp):
    bf = ml_dtypes.bfloat16
    b, h = divmod(c, 2)
    sc = float(E) ** -0.5
    own = slice(512 * h, 512 * h + 512)
    oth = slice(512 * (1 - h), 512 * (1 - h) + 512)

    def stack_heads(w):  # [16, E, D] -> [E, 1024]
        return np.ascontiguousarray(np.transpose(w, (1, 0, 2)).reshape(E, E))

    xt = np.asarray(inp["x"][b], np.float32)           # [T, E]
    xperm = np.concatenate([xt[own], xt[oth]], axis=0)  # keys permuted: own first
    # own-half causal mask (key blocks 0..3 in permuted order): sp <= f
    sp, f = np.arange(512), np.arange(512)
    mask = (sp[:, None] <= f[None, :])                  # [512, 512]
    smask = mask.reshape(4, 128, 512).transpose(1, 0, 2).astype(bf)

    return {
        "xTb": np.ascontiguousarray(xperm.T).astype(bf),
        "xTo": np.ascontiguousarray(xt[own].T).astype(np.float32),
        "caT": np.ascontiguousarray(np.asarray(inp["ca"][b]).T).astype(bf),
        "wq": (stack_heads(inp["Wq_s"]) * sc).astype(bf),
        "wk": stack_heads(inp["Wk_s"]).astype(bf),
        "wv": stack_heads(inp["Wv_s"]).astype(bf),
        "wqc": (stack_heads(inp["Wq_c"]) * sc).astype(bf),
        "wkc": stack_heads(inp["Wk_c"]).astype(bf),
        "wvc": stack_heads(inp["Wv_c"]).astype(bf),
        "wo": np.asarray(inp["Wo_s"], np.float32).astype(bf),
        "woc": np.asarray(inp["Wo_c"], np.float32).astype(bf),
        "w1": np.asarray(inp["W1"], np.float32).astype(bf),
        "w2": np.asarray(inp["W2"], np.float32).astype(bf),
        "g1": np.asarray(inp["ln1_g"], np.float32),
        "g2": np.asarray(inp["ln2_g"], np.float32),
        "g3": np.asarray(inp["ln3_g"], np.float32),
        "bl1": np.asarray(inp["ln1_b"], np.float32),
        "bl2": np.asarray(inp["ln2_b"], np.float32),
        "bl3": np.asarray(inp["ln3_b"], np.float32),
        "bo": np.asarray(inp["bo_s"], np.float32),
        "boc": np.asarray(inp["bo_c"], np.float32),
        "b2": np.asarray(inp["b2"], np.float32),
        "b1r": np.asarray(inp["b1"], np.float32),
        "smask": smask,
        "oflag": np.full((128, 1), float(h), np.float32),
    }



# revision 41
# speedup vs baseline: 1.8296x; 1.8296x over previous
"""Trainium2 Bass kernel for nn_DecoderBlock (B=4,T=S=E=1024,H=16,D=64) on 8 cores.

Sharding: data-parallel over batch B=4 x tensor-parallel 2 (heads 16->8,
FFN hidden 4096->2048), AllReduce between core pairs after each projection.

Layout trick: the whole residual stream is kept TRANSPOSED on-chip as
x^T [E(partitions), T(free)] so every matmul's operands are already in the
natural lhsT/rhs layout and no PE transposes are needed.  LayerNorm reduces
over the partition dim via ones-matmuls; the per-token mean/rstd rows are
broadcast back across partitions with tiny K=1 / K=2 matmuls.
Softmax denominators come for free from an extra ones-column appended to V.
"""
import sys

sys.path.insert(0, "/opt/trn_rl_repo")

import numpy as np
import ml_dtypes

import concourse.bass as bass
import concourse.bacc as bacc
import concourse.mybir as mybir
import concourse.tile as tile

BF16 = mybir.dt.bfloat16
F32 = mybir.dt.float32
F8 = mybir.dt.float8e4
DR = mybir.MatmulPerfMode.DoubleRow
AF = mybir.ActivationFunctionType
OP = mybir.AluOpType

W8 = 64.0                      # fp8 weight pre-scale (keeps e4m3 in normal range)
SC8 = 1.0 / (W8 * W8)          # undo two fp8 weight pre-scales

B, T, S, E, H, D = 4, 1024, 1024, 1024, 16, 64
HL = H // 2          # heads per core (TP-2)
FF = 4 * E // 2      # ffn hidden per core
KO = E // 128        # 8 partition subtiles of E
NC_ = 512            # matmul free-dim chunk
CC = T // NC_        # 2 chunks over T
PAIRS = [[0, 1], [2, 3], [4, 5], [6, 7]]


SKIP_CC = False


def build(nbody=1):
    nc = bacc.Bacc(num_devices=8)

    def P(name, shape, dt):
        return nc.declare_dram_parameter(name, shape, dt, isOutput=False)

    xT = P("xT", [E, T], F32)
    caT = P("caT", [E, S], BF16)
    wq, wk, wv = P("wq", [E, 512], BF16), P("wk", [E, 512], BF16), P("wv", [E, 512], BF16)
    wqc, wkc, wvc = P("wqc", [E, 512], BF16), P("wkc", [E, 512], BF16), P("wvc", [E, 512], BF16)
    wo, woc = P("wo", [512, E], BF16), P("woc", [512, E], BF16)
    w1, w2 = P("w1", [E, FF], BF16), P("w2", [FF, E], BF16)
    gb = [P(f"gb{i}", [2, E], BF16) for i in (1, 2, 3)]
    gpp_d = [P(f"g{i}", [E], F32) for i in (1, 2, 3)]
    bo2, bo2c, b22 = P("bo2", [E], F32), P("bo2c", [E], F32), P("b22", [E], F32)
    b1r = P("b1r", [FF], F32)
    cmask = P("cmask", [128, 4, 512], BF16)
    out_xT = nc.declare_dram_parameter("out_xT", [E, T], F32, isOutput=True)

    with tile.TileContext(nc) as tc:
        with tc.tile_pool(name="persist", bufs=1) as pp:
            xT_sb = pp.tile([128, KO, T], F32, tag="xT")
            for ko in range(KO):
                nc.sync.dma_start(out=xT_sb[:, ko, :],
                                  in_=xT[ko * 128:(ko + 1) * 128, :])
            ca_sb = pp.tile([128, KO, S], BF16, tag="ca")
            nc.sync.dma_start(out=ca_sb[:], in_=caT.rearrange("(ko p) t -> p ko t", p=128))
            cm_sb = pp.tile([128, 4, 512], BF16, tag="cm")
            nc.sync.dma_start(out=cm_sb[:], in_=cmask[:])
            ones_bf = pp.tile([128, 512], BF16, tag="ones")
            nc.vector.memset(ones_bf[:], 1.0)
            gl_sb, bl_sb, gpp = [], [], []
            for i in range(3):
                ta = pp.tile([1, KO, 128], BF16, tag=f"gl{i}")
                nc.sync.dma_start(out=ta[:], in_=gb[i].rearrange("a (ko m) -> a ko m", m=128)[0:1])
                gl_sb.append(ta)
                tb = pp.tile([1, KO, 128], BF16, tag=f"bl{i}")
                nc.sync.dma_start(out=tb[:], in_=gb[i].rearrange("a (ko m) -> a ko m", m=128)[1:2])
                bl_sb.append(tb)
                t2 = pp.tile([128, KO], F32, tag=f"gpp{i}")
                with nc.allow_non_contiguous_dma(reason="tiny LN vector"):
                    nc.sync.dma_start(out=t2[:], in_=gpp_d[i].rearrange("(ko p) -> p ko", p=128))
                gpp.append(t2)
            bpp = []
            for nm, d in (("bo2", bo2), ("bo2c", bo2c), ("b22", b22)):
                t_ = pp.tile([128, KO], F32, tag=nm)
                with nc.allow_non_contiguous_dma(reason="tiny bias vector"):
                    nc.sync.dma_start(out=t_[:], in_=d.rearrange("(ko p) -> p ko", p=128))
                bpp.append(t_)
            eps_t = pp.tile([1, 1], F32, tag="eps")
            nc.vector.memset(eps_t[:], 1e-5)
            b1pp = pp.tile([128, FF // 128], F32, tag="b1")
            with nc.allow_non_contiguous_dma(reason="tiny bias vector"):
                nc.sync.dma_start(out=b1pp[:], in_=b1r.rearrange("(m p) -> p m", p=128))

            for ibody in range(nbody):
                _body(nc, tc, ibody, xT_sb, ca_sb, cm_sb, ones_bf, (gl_sb, bl_sb), gpp,
                      bpp, b1pp, eps_t,
                      dict(wq=wq, wk=wk, wv=wv, wqc=wqc, wkc=wkc, wvc=wvc,
                           wo=wo, woc=woc, w1=w1, w2=w2, xT=xT),
                      out_xT)
    nc.finalize()
    return nc


def _body(nc, tc, ibody, xT_sb, ca_sb, cm_sb, ones_bf, gbl, gpp,
          bpp, b1pp, eps_t, W, out_xT):
    gl_sb, bl_sb = gbl
    bo2pp, bo2cpp, b22pp = bpp
    ar = {}
    for k in (1, 2, 3):
        ar[k] = [(nc.dram_tensor(f"ar{k}_{ibody}_{c}_in", [E, NC_], F32),
                  nc.dram_tensor(f"ar{k}_{ibody}_{c}_out", [E, NC_], F32))
                 for c in range(CC)]

    if ibody > 0:
        # re-load pristine x for the timing replica
        for ko in range(KO):
            nc.sync.dma_start(out=xT_sb[:, ko, :],
                              in_=W["xT"][ko * 128:(ko + 1) * 128, :])

    with tc.tile_pool(name=f"A{ibody}", bufs=1) as pa, \
         tc.tile_pool(name=f"ps{ibody}", bufs=8, space="PSUM") as pspool:

        def ps():
            return pspool.tile([128, NC_], F32, tag="ps", name="ps")

        def layer_norm(i):
            """LN over partitions of xT_sb -> bf16 tile [128, KO, T]."""
            ln = pa.tile([128, KO, T], BF16, tag="lnout", name="ln")
            for c in range(CC):
                cs = slice(c * NC_, (c + 1) * NC_)
                xb = pa.tile([128, KO, NC_], BF16, tag="stat", bufs=2, name="xb")
                for ko in range(KO):
                    nc.scalar.copy(out=xb[:, ko, :], in_=xT_sb[:, ko, cs])
                sq = pa.tile([128, KO, NC_], BF16, tag="stat", bufs=2, name="sq")
                nc.scalar.activation(sq[:], xb[:], AF.Square)
                ps1, ps2 = ps(), ps()
                for ko in range(KO):
                    nc.tensor.matmul(ps1[0:1, :], ones_bf[:, 0:1], xb[:, ko, :],
                                     start=(ko == 0), stop=(ko == KO - 1))
                for ko in range(KO):
                    nc.tensor.matmul(ps2[0:1, :], ones_bf[:, 0:1], sq[:, ko, :],
                                     start=(ko == 0), stop=(ko == KO - 1))
                m_ = pa.tile([1, NC_], F32, tag="row_m", bufs=1, name="m_")
                nc.vector.tensor_scalar_mul(m_[:], ps1[0:1, :], 1.0 / E)
                msq = pa.tile([1, NC_], F32, tag="row_q", bufs=1, name="msq")
                nc.vector.tensor_mul(msq[:], m_[:], m_[:])
                var = pa.tile([1, NC_], F32, tag="row_v", bufs=1, name="var")
                nc.vector.scalar_tensor_tensor(var[:], ps2[0:1, :], 1.0 / E,
                                               msq[:], OP.mult, OP.subtract)
                sqv = pa.tile([1, NC_], F32, tag="row_s", bufs=1, name="sqv")
                nc.scalar.activation(sqv[:], var[:], AF.Sqrt, bias=eps_t[:])
                rstd = pa.tile([1, NC_], F32, tag="row_r", bufs=1, name="rstd")
                nc.vector.reciprocal(rstd[:], sqv[:])
                rbf = pa.tile([1, NC_], BF16, tag="rowsb2", bufs=1, name="rbf")
                nc.vector.tensor_copy(rbf[:], rstd[:])
                nmr = pa.tile([1, NC_], BF16, tag="rowsb1", bufs=1, name="nmr")
                # nmr = -m * rstd
                nc.vector.scalar_tensor_tensor(nmr[:], m_[:], -1.0,
                                               rstd[:], OP.mult, OP.mult)
                rbc = ps()
                nc.tensor.matmul(rbc[:, :], ones_bf[0:1, 0:128], rbf[:],
                                 start=True, stop=True)
                for ko in range(KO):
                    bbc = ps()
                    nc.tensor.matmul(bbc[:, :], gl_sb[i][:, ko, :], nmr[:],
                                     start=True, stop=False)
                    nc.tensor.matmul(bbc[:, :], bl_sb[i][:, ko, :],
                                     ones_bf[0:1, 0:NC_], start=False, stop=True)
                    t0 = pa.tile([128, NC_], F32, tag="tmp", bufs=2, name="t0")
                    nc.vector.scalar_tensor_tensor(t0[:], xT_sb[:, ko, cs],
                                                   gpp[i][:, ko:ko + 1], rbc[:, :],
                                                   OP.mult, OP.mult)
                    nc.vector.tensor_tensor(ln[:, ko, cs], t0[:], bbc[:, :], OP.add)
            return ln

        def project_qk(pb_, lnsrc, w_d, tag, bufs=1):
            """-> [128, 4, T] bf16 : rows = 2 heads x 64, per pair j."""
            w_sb = pb_.tile([128, KO, 512], BF16, tag="wqkv", bufs=2, name="wsb")
            nc.sync.dma_start(out=w_sb[:], in_=w_d.rearrange("(ko p) m -> p ko m", p=128))
            qt = pb_.tile([128, 4, T], BF16, tag=tag, bufs=bufs, name="qt")
            for j in range(4):
                for c in range(CC):
                    p_ = ps()
                    for ko in range(KO):
                        nc.tensor.matmul(p_[:, :], w_sb[:, ko, j * 128:(j + 1) * 128],
                                         lnsrc[:, ko, c * NC_:(c + 1) * NC_],
                                         start=(ko == 0), stop=(ko == KO - 1))
                    nc.vector.tensor_copy(qt[:, j, c * NC_:(c + 1) * NC_], p_[:, :])
            return qt

        def project_v(pb_, src, w_d):
            """-> [128, 8, 8, 65] bf16 : [s_part, s_sub, head, d|ones]."""
            w_sb = pb_.tile([128, KO, 512], BF16, tag="wqkv", bufs=2, name="wsb")
            nc.sync.dma_start(out=w_sb[:], in_=w_d.rearrange("(ko p) m -> p ko m", p=128))
            vv = pb_.tile([128, 8, HL, 65], BF16, tag="vv", name="vv")
            for s in range(8):
                p_ = ps()
                for ko in range(KO):
                    nc.tensor.matmul(p_[:, :], src[:, ko, s * 128:(s + 1) * 128],
                                     w_sb[:, ko, :], start=(ko == 0), stop=(ko == KO - 1))
                nc.scalar.copy(out=vv[:, s, :, 0:64],
                               in_=p_[:, :].rearrange("p (h d) -> p h d", d=64))
                nc.vector.memset(vv[:, s, :, 64:65], 1.0)
            return vv

        def attention(pb_, qt, kt, vv, causal):
            onorm = pb_.tile([128, 4, T], BF16, tag="onorm", name="onorm")
            for c in range(CC):
                for h in range(HL):
                    j, half = h // 2, h % 2
                    pb = 64 * half
                    subs = list(range(4 * (c + 1))) if causal else list(range(8))
                    eb = pb_.tile([128, 8, NC_], BF16, tag="expb", bufs=2, name="eb")
                    for s_ in subs:
                        p_ = ps()
                        nc.tensor.matmul(p_[:, :],
                                         kt[pb:pb + 64, j, s_ * 128:(s_ + 1) * 128],
                                         qt[pb:pb + 64, j, c * NC_:(c + 1) * NC_],
                                         start=True, stop=True)
                        nc.scalar.activation(eb[:, s_, :], p_[:, :], AF.Exp)
                        if causal and s_ >= 4 * c:
                            nc.vector.tensor_mul(eb[:, s_, :], eb[:, s_, :],
                                                 cm_sb[:, s_ - 4 * c, :])
                    av = ps()
                    for i_, s_ in enumerate(subs):
                        nc.tensor.matmul(av[0:65, :], vv[:, s_, h, :], eb[:, s_, :],
                                         start=(i_ == 0), stop=(i_ == len(subs) - 1))
                    rr = pb_.tile([65, NC_], F32, tag="row_rr", bufs=2, name="rr")
                    nc.vector.reciprocal(rr[64:65, :], av[64:65, :])
                    rb = pb_.tile([65, NC_], BF16, tag="row_rrb", bufs=2, name="rb")
                    nc.vector.tensor_copy(rb[64:65, :], rr[64:65, :])
                    bc = ps()
                    nc.tensor.matmul(bc[0:64, :], ones_bf[64:65, 0:64], rb[64:65, :],
                                     start=True, stop=True)
                    bcs = pb_.tile([64, NC_], BF16, tag="bcs", bufs=2, name="bcs")
                    nc.vector.tensor_copy(bcs[:, :], bc[0:64, :])
                    nc.vector.tensor_tensor(onorm[pb:pb + 64, j, c * NC_:(c + 1) * NC_],
                                            av[0:64, :], bcs[:, :], OP.mult)
            return onorm

        def out_proj(pb_, onorm, wo_d, ark, bias_pp):
            wo_sb = pb_.tile([128, 4, E], BF16, tag="wo", name="wo_sb")
            nc.sync.dma_start(out=wo_sb[:], in_=wo_d.rearrange("(ks p) e -> p ks e", p=128))
            for c in range(CC):
                for m in range(KO):
                    p_ = ps()
                    for ks in range(4):
                        nc.tensor.matmul(p_[:, :], wo_sb[:, ks, m * 128:(m + 1) * 128],
                                         onorm[:, ks, c * NC_:(c + 1) * NC_],
                                         start=(ks == 0), stop=(ks == 3))
                    st = pa.tile([128, NC_], F32, tag="arst", bufs=4, name="st")
                    nc.vector.tensor_scalar_add(st[:, :], p_[:, :], bias_pp[:, m:m + 1])
                    nc.sync.dma_start(out=ark[c][0][m * 128:(m + 1) * 128, :], in_=st[:])
                allreduce_c(ark, c)

        def allreduce_c(ark, c):
            a_in, a_out = ark[c]
            if SKIP_CC:
                nc.sync.dma_start(out=a_out[:], in_=a_in[:])
            else:
                nc.gpsimd.collective_compute(
                    "AllReduce", OP.add, replica_groups=PAIRS,
                    ins=[a_in[:]], outs=[a_out[:]])
            nc.gpsimd.dma_start(
                out=xT_sb[:, :, c * NC_:(c + 1) * NC_],
                in_=a_out.rearrange("(ko p) t -> p ko t", p=128),
                accum_op=OP.add)



        with tc.tile_pool(name=f"B{ibody}", bufs=1) as pb_:
            # ---- self attention ----
            ln1 = layer_norm(0)
            qt = project_qk(pb_, ln1, W["wq"], "qt")
            kt = project_qk(pb_, ln1, W["wk"], "kt")
            vv = project_v(pb_, ln1, W["wv"])
            on1 = attention(pb_, qt, kt, vv, causal=True)
            out_proj(pb_, on1, W["wo"], ar[1], bo2pp)
            # cross K/V from raw ca — independent of AR1, fills the gap
            ktc = project_qk(pb_, ca_sb, W["wkc"], "kt")
            vvc = project_v(pb_, ca_sb, W["wvc"])

            # ---- cross attention ----
            ln2 = layer_norm(1)
            qtc = project_qk(pb_, ln2, W["wqc"], "qt")
            on2 = attention(pb_, qtc, ktc, vvc, causal=False)
            out_proj(pb_, on2, W["woc"], ar[2], bo2cpp)

        # ---- FFN ----
        ln3 = layer_norm(2)
        with tc.tile_pool(name=f"C{ibody}", bufs=1) as pc:
            ht = pc.tile([128, FF // 128, T], BF16, tag="ht", name="ht")
            for m in range(FF // 128):
                w1m = pc.tile([128, KO, 128], BF16, tag="w1m", bufs=4, name="w1m")
                nc.sync.dma_start(
                    out=w1m[:],
                    in_=W["w1"].rearrange("(ko p) f -> p ko f", p=128)[:, :, m * 128:(m + 1) * 128])
                for c in range(CC):
                    p_ = ps()
                    for ko in range(KO):
                        nc.tensor.matmul(p_[:, :], w1m[:, ko, :],
                                         ln3[:, ko, c * NC_:(c + 1) * NC_],
                                         start=(ko == 0), stop=(ko == KO - 1))
                    nc.scalar.activation(ht[:, m, c * NC_:(c + 1) * NC_], p_[:, :],
                                         AF.Relu, bias=b1pp[:, m:m + 1])
            w2m_t = [None] * KO
            for m in range(KO):
                w2m = pc.tile([128, FF // 128, 128], BF16, tag="w2m", bufs=8, name="w2m")
                nc.sync.dma_start(
                    out=w2m[:],
                    in_=W["w2"].rearrange("(ks p) e -> p ks e", p=128)[:, :, m * 128:(m + 1) * 128])
                w2m_t[m] = w2m
            for c in range(CC):
                for m in range(KO):
                    p_ = ps()
                    for ks in range(FF // 128):
                        nc.tensor.matmul(p_[:, :], w2m_t[m][:, ks, :],
                                         ht[:, ks, c * NC_:(c + 1) * NC_],
                                         start=(ks == 0), stop=(ks == FF // 128 - 1))
                    st = pa.tile([128, NC_], F32, tag="arst", bufs=4, name="st")
                    nc.vector.tensor_scalar_add(st[:, :], p_[:, :], b22pp[:, m:m + 1])
                    nc.sync.dma_start(out=ar[3][c][0][m * 128:(m + 1) * 128, :], in_=st[:])
                allreduce_c(ar[3], c)
                for ko in range(KO):
                    nc.sync.dma_start(
                        out=out_xT[ko * 128:(ko + 1) * 128, c * NC_:(c + 1) * NC_],
                        in_=xT_sb[:, ko, c * NC_:(c + 1) * NC_])


# ------------------------------------------------------------------ host side

_CACHE = {}


COMM_FREE = True


def _get_runner(nbody=1, loop_iters=None):
    key = (nbody, loop_iters, COMM_FREE)
    if key in _CACHE:
        return _CACHE[key]
    import jax
    from jax.sharding import Mesh, PartitionSpec
    from jax.experimental.shard_map import shard_map
    from concourse.bass2jax import (_bass_exec_p, install_neuronx_cc_hook,
                                    partition_id_tensor)

    nc = (build_nocc(nbody, loop_iters) if COMM_FREE else build(nbody))
    install_neuronx_cc_hook()
    pn = nc.partition_id_tensor.name if nc.partition_id_tensor else None
    in_names, out_names, out_avals = [], [], []
    for alloc in nc.m.functions[0].allocations:
        if not isinstance(alloc, mybir.MemoryLocationSet):
            continue
        name = alloc.memorylocations[0].name
        if alloc.kind == "ExternalInput":
            if name != pn:
                in_names.append(name)
        elif alloc.kind == "ExternalOutput":
            out_names.append(name)
            out_avals.append(jax.core.ShapedArray(
                tuple(alloc.tensor_shape), mybir.dt.np(alloc.dtype)))
    n_params = len(in_names)
    all_in = in_names + out_names + ([pn] if pn else [])

    def _jbody(*args):
        ops = list(args)
        if pn:
            ops.append(partition_id_tensor())
        return tuple(_bass_exec_p.bind(
            *ops, out_avals=tuple(out_avals), in_names=tuple(all_in),
            out_names=tuple(out_names), lowering_input_output_aliases=(),
            sim_require_finite=True, sim_require_nnan=True, nc=nc))

    devices = jax.devices()[:8]
    mesh = Mesh(np.asarray(devices), ("core",))
    spec = (PartitionSpec("core"),)
    fn = jax.jit(shard_map(_jbody, mesh=mesh,
                           in_specs=spec * (n_params + len(out_names)),
                           out_specs=spec * len(out_names), check_rep=False),
                 keep_unused=True)
    _CACHE[key] = (fn, in_names, out_names, out_avals)
    return _CACHE[key]


def _get_loop_runner(niter):
    return _get_runner(1, loop_iters=niter)


def _make_core_inputs(c, inp):
    bf = ml_dtypes.bfloat16
    b, r = divmod(c, 2)
    hs = slice(8 * r, 8 * r + 8)
    sc = float(E) ** -0.5

    def stack_heads(w):  # [8, E, D] -> [E, 512]
        return np.ascontiguousarray(np.transpose(w, (1, 0, 2)).reshape(E, 512))

    p, f = np.arange(128)[:, None, None], np.arange(512)[None, None, :]
    jj = np.arange(4)[None, :, None]
    cmask = (f >= 128 * jj + p).astype(bf)

    return {
        "xT": np.ascontiguousarray(inp["x"][b].T).astype(np.float32),
        "caT": np.ascontiguousarray(inp["ca"][b].T).astype(bf),
        "wq": (stack_heads(inp["Wq_s"][hs]) * sc).astype(bf),
        "wk": stack_heads(inp["Wk_s"][hs]).astype(bf),
        "wv": stack_heads(inp["Wv_s"][hs]).astype(bf),
        "wqc": (stack_heads(inp["Wq_c"][hs]) * sc).astype(bf),
        "wkc": stack_heads(inp["Wk_c"][hs]).astype(bf),
        "wvc": stack_heads(inp["Wv_c"][hs]).astype(bf),
        "wo": np.ascontiguousarray(inp["Wo_s"][512 * r:512 * (r + 1), :]).astype(bf),
        "woc": np.ascontiguousarray(inp["Wo_c"][512 * r:512 * (r + 1), :]).astype(bf),
        "w1": np.ascontiguousarray(inp["W1"][:, FF * r:FF * (r + 1)]).astype(bf),
        "w2": np.ascontiguousarray(inp["W2"][FF * r:FF * (r + 1), :]).astype(bf),
        "gb1": np.stack([inp["ln1_g"], inp["ln1_b"]]).astype(bf),
        "gb2": np.stack([inp["ln2_g"], inp["ln2_b"]]).astype(bf),
        "gb3": np.stack([inp["ln3_g"], inp["ln3_b"]]).astype(bf),
        "g1": np.asarray(inp["ln1_g"], np.float32),
        "g2": np.asarray(inp["ln2_g"], np.float32),
        "g3": np.asarray(inp["ln3_g"], np.float32),
        "bo2": np.asarray(inp["bo_s"], np.float32) * 0.5,
        "bo2c": np.asarray(inp["bo_c"], np.float32) * 0.5,
        "b22": np.asarray(inp["b2"], np.float32) * 0.5,
        "b1r": np.asarray(inp["b1"][FF * r:FF * (r + 1)], np.float32),
        "cmask": cmask,
    }


def _run(nbody, in_maps, dev_inputs=None, dev_zeros=None, download=True,
         loop_iters=None):
    import jax
    fn, in_names, out_names, out_avals = _get_runner(nbody, loop_iters)
    if dev_inputs is None:
        concat = [np.concatenate([np.asarray(in_maps[c][n]) for c in range(8)], axis=0)
                  for n in in_names]
        dev_inputs = [jax.device_put(a) for a in concat]
    if dev_zeros is None:
        dev_zeros = [jax.device_put(np.zeros((8 * a.shape[0], *a.shape[1:]), a.dtype))
                     for a in out_avals]
    outs = fn(*dev_inputs, *dev_zeros)
    for o in outs:
        o.block_until_ready()
    if not download:
        return None, (dev_inputs, dev_zeros)
    res = []
    for c in range(8):
        res.append({n: np.asarray(outs[i]).reshape(8, *out_avals[i].shape)[c]
                    for i, n in enumerate(out_names)})
    return res, (dev_inputs, dev_zeros)


def kernel(**inputs):
    inp = {k: np.asarray(v) for k, v in inputs.items()}
    mk = _make_core_inputs_nocc if COMM_FREE else _make_core_inputs
    in_maps = [mk(c, inp) for c in range(8)]
    res, _ = _run(1, in_maps, loop_iters=1 if COMM_FREE else None)
    if COMM_FREE:
        out = np.stack([
            np.concatenate([res[2 * b]["out_xT"], res[2 * b + 1]["out_xT"]],
                           axis=1).T
            for b in range(B)]).astype(np.float32)
    else:
        out = np.stack([res[2 * b]["out_xT"].T for b in range(B)]).astype(np.float32)
    return out


# ---------------------------------------------------------------- comm-free

def build_nocc(nbody=1, loop_iters=None):
    """Communication-free sharding: core = (batch b, T-half h).  Each core
    computes its 512 query tokens for ALL 16 heads and the full FFN, with
    K/V duplicated across the pair.  Self-attn keys are permuted so the own
    half always sits at key positions 0..511 (the per-core causal mask input
    encodes the permutation) — keeps the SPMD program identical on all cores.
    """
    nc = bacc.Bacc(num_devices=8)

    def P(name, shape, dt):
        return nc.declare_dram_parameter(name, shape, dt, isOutput=False)

    xTb = P("xTb", [E, T], BF16)        # permuted x^T, bf16 (LN1 / self K,V)
    xTo = P("xTo", [E, NC_], F32)       # own-half x^T, f32 (residual base)
    caT = P("caT", [E, S], BF16)
    wq, wk, wv = P("wq", [E, E], BF16), P("wk", [E, E], BF16), P("wv", [E, E], BF16)
    wqc, wkc, wvc = P("wqc", [E, E], BF16), P("wkc", [E, E], BF16), P("wvc", [E, E], BF16)
    wo, woc = P("wo", [E, E], BF16), P("woc", [E, E], BF16)
    w1, w2 = P("w1", [E, 4 * E], BF16), P("w2", [4 * E, E], BF16)
    gpp_d = [P(f"g{i}", [E], F32) for i in (1, 2, 3)]
    blp_d = [P(f"bl{i}", [E], F32) for i in (1, 2, 3)]
    bo_, boc_, b2_ = P("bo", [E], F32), P("boc", [E], F32), P("b2", [E], F32)
    b1r = P("b1r", [4 * E], F32)
    smask = P("smask", [128, 4, NC_], BF16)
    oflag = P("oflag", [128, 1], F32)
    out_xT = nc.declare_dram_parameter("out_xT", [E, NC_], F32, isOutput=True)

    with tile.TileContext(nc) as tc:
        with tc.tile_pool(name="persist", bufs=1) as pp:
            xTb_sb = pp.tile([128, KO, T], BF16, tag="xTb")
            for ko in range(KO):
                nc.sync.dma_start(out=xTb_sb[:, ko, :], in_=xTb[ko * 128:(ko + 1) * 128, :])
            xTo_sb = pp.tile([128, KO, NC_], F32, tag="xTo")
            nc.sync.dma_start(out=xTo_sb[:], in_=xTo.rearrange("(ko p) t -> p ko t", p=128))
            ca_sb = pp.tile([128, KO, S], BF16, tag="ca")
            nc.sync.dma_start(out=ca_sb[:], in_=caT.rearrange("(ko p) t -> p ko t", p=128))
            sm_sb = pp.tile([128, 4, NC_], BF16, tag="sm")
            nc.sync.dma_start(out=sm_sb[:], in_=smask[:])
            ones_bf = pp.tile([128, 512], BF16, tag="ones")
            nc.vector.memset(ones_bf[:], 1.0)
            fl_sb = pp.tile([128, 1], F32, tag="oflag")
            with nc.allow_non_contiguous_dma(reason="tiny flag vector"):
                nc.sync.dma_start(out=fl_sb[:], in_=oflag[:])
            gpp, blpp = [], []
            for i in range(3):
                t2 = pp.tile([128, KO], F32, tag=f"gpp{i}")
                with nc.allow_non_contiguous_dma(reason="tiny LN vector"):
                    nc.sync.dma_start(out=t2[:], in_=gpp_d[i].rearrange("(ko p) -> p ko", p=128))
                gpp.append(t2)
                t3 = pp.tile([128, KO], F32, tag=f"blpp{i}")
                with nc.allow_non_contiguous_dma(reason="tiny LN vector"):
                    nc.sync.dma_start(out=t3[:], in_=blp_d[i].rearrange("(ko p) -> p ko", p=128))
                blpp.append(t3)
            bpp = []
            for nm, d in (("bo", bo_), ("boc", boc_), ("b2", b2_)):
                t_ = pp.tile([128, KO], F32, tag=nm)
                with nc.allow_non_contiguous_dma(reason="tiny bias vector"):
                    nc.sync.dma_start(out=t_[:], in_=d.rearrange("(ko p) -> p ko", p=128))
                bpp.append(t_)
            eps_t = pp.tile([1, 1], F32, tag="eps")
            nc.vector.memset(eps_t[:], 1e-5)
            b1pp = pp.tile([128, 4 * E // 128], F32, tag="b1")
            with nc.allow_non_contiguous_dma(reason="tiny bias vector"):
                nc.sync.dma_start(out=b1pp[:], in_=b1r.rearrange("(m p) -> p m", p=128))

            Wd = dict(wq=wq, wk=wk, wv=wv, wqc=wqc, wkc=wkc, wvc=wvc,
                      wo=wo, woc=woc, w1=w1, w2=w2, xTo=xTo)
            if loop_iters is None:
                for ibody in range(nbody):
                    _body_nocc(nc, tc, ibody, xTb_sb, xTo_sb, ca_sb, sm_sb,
                               ones_bf, fl_sb, (gpp, blpp), bpp, b1pp, eps_t,
                               Wd, out_xT)
            else:
                with tc.For_i(0, loop_iters, 1):
                    _body_nocc(nc, tc, 1, xTb_sb, xTo_sb, ca_sb, sm_sb,
                               ones_bf, fl_sb, (gpp, blpp), bpp, b1pp, eps_t,
                               Wd, out_xT)
    nc.finalize()
    return nc


def _body_nocc(nc, tc, ibody, xTb_sb, xTo_sb, ca_sb, sm_sb, ones_bf, fl_sb, gbl,
               bpp, b1pp, eps_t, W, out_xT):
    gpp, blpp = gbl
    bopp, bocpp, b2pp = bpp

    if ibody > 0:
        nc.sync.dma_start(out=xTo_sb[:],
                          in_=W["xTo"].rearrange("(ko p) t -> p ko t", p=128))

    with tc.tile_pool(name=f"A{ibody}", bufs=1) as pa, \
         tc.tile_pool(name=f"ps{ibody}", bufs=8, space="PSUM") as pspool:

        pb2_ref = [None]

        def ps():
            return pspool.tile([128, NC_], F32, tag="ps", bufs=4, name="ps")

        def psw():
            return pspool.tile([128, 2 * NC_], F32, tag="pw", bufs=2, name="pw")

        def ln_rows(i, ps1, ps2, cs_out, ln, src, src_is_bf, gsl, ncols):
            m_ = pa.tile([1, NC_], F32, tag="row_m", bufs=1, name="m_")
            nc.vector.tensor_scalar_mul(m_[:, :ncols], ps1[0:1, :ncols], 1.0 / E)
            msq = pa.tile([1, NC_], F32, tag="row_q", bufs=1, name="msq")
            nc.vector.tensor_mul(msq[:, :ncols], m_[:, :ncols], m_[:, :ncols])
            var = pa.tile([1, NC_], F32, tag="row_v", bufs=1, name="var")
            nc.vector.scalar_tensor_tensor(var[:, :ncols], ps2[0:1, :ncols], 1.0 / E,
                                           msq[:, :ncols], OP.mult, OP.subtract)
            sqv = pa.tile([1, NC_], F32, tag="row_s", bufs=1, name="sqv")
            nc.scalar.activation(sqv[:, :ncols], var[:, :ncols], AF.Sqrt, bias=eps_t[:])
            rbf = pa.tile([1, NC_], BF16, tag="rowsb2", bufs=1, name="rbf")
            with nc.allow_low_precision(reason="rstd rounds to bf16 anyway"):
                nc.vector.reciprocal(rbf[:, :ncols], sqv[:, :ncols])
            nmr = pa.tile([1, NC_], BF16, tag="rowsb1", bufs=1, name="nmr")
            nc.vector.scalar_tensor_tensor(nmr[:, :ncols], m_[:, :ncols], -1.0,
                                           rbf[:, :ncols], OP.mult, OP.mult)
            rbc = ps()
            nc.tensor.matmul(rbc[:, :ncols], ones_bf[0:1, 0:128], rbf[:, :ncols],
                             start=True, stop=True)
            nmb = ps()
            nc.tensor.matmul(nmb[:, :ncols], ones_bf[0:1, 0:128], nmr[:, :ncols],
                             start=True, stop=True)
            for ko in range(KO):
                bbc = pa.tile([128, NC_], BF16, tag="bbc", bufs=2, name="bbc")
                nc.scalar.activation(bbc[:, :ncols], nmb[:, :ncols], AF.Identity,
                                     bias=blpp[i][:, ko:ko + 1],
                                     scale=gpp[i][:, ko:ko + 1])
                t0 = pa.tile([128, NC_], F32, tag="tmp", bufs=2, name="t0")
                nc.vector.scalar_tensor_tensor(t0[:, :ncols], src[ko],
                                               gpp[i][:, ko:ko + 1], rbc[:, :ncols],
                                               OP.mult, OP.mult)
                with nc.allow_low_precision(reason="ln output feeds fp8 matmuls"):
                    nc.vector.tensor_tensor(ln[:, ko, cs_out], t0[:, :ncols],
                                            bbc[:, :ncols], OP.add)

        def layer_norm1():
            """full-T LN over xTb (bf16 source)."""
            ln = pa.tile([128, KO, T], BF16, tag="lnf", name="lnf")
            for c in range(CC):
                cs = slice(c * NC_, (c + 1) * NC_)
                sq = pa.tile([128, KO, NC_], BF16, tag="stat", bufs=2, name="sq")
                nc.scalar.activation(sq[:], xTb_sb[:, :, cs], AF.Square)
                ps1, ps2 = ps(), ps()
                for ko in range(KO):
                    nc.tensor.matmul(ps1[0:1, :], ones_bf[:, 0:1], xTb_sb[:, ko, cs],
                                     start=(ko == 0), stop=(ko == KO - 1))
                for ko in range(KO):
                    nc.tensor.matmul(ps2[0:1, :], ones_bf[:, 0:1], sq[:, ko, :],
                                     start=(ko == 0), stop=(ko == KO - 1))
                ln_rows(0, ps1, ps2, cs, ln,
                        [xTb_sb[:, ko, cs] for ko in range(KO)], True, None, NC_)
            return ln

        def ln_stats_step(st, ko):
            """accumulate own-half LN stats for one ko sub-tile of xTo."""
            xb, sq, ps1, ps2 = st
            nc.scalar.copy(out=xb[:, ko, :], in_=xTo_sb[:, ko, :])
            nc.scalar.activation(sq[:, ko, :], xb[:, ko, :], AF.Square)
            nc.tensor.matmul(ps1[0:1, :], ones_bf[:, 0:1], xb[:, ko, :],
                             start=(ko == 0), stop=(ko == KO - 1))
            nc.tensor.matmul(ps2[0:1, :], ones_bf[:, 0:1], sq[:, ko, :],
                             start=(ko == 0), stop=(ko == KO - 1))

        def layer_norm_h_rows(i, st, dt=BF16, tag="lnh"):
            """own-half LN rows + normalize, after ln_stats_steps are done."""
            ln = pa.tile([128, KO, NC_], dt, tag=tag, bufs=1, name="lnh")
            ln_rows(i, st[2], st[3], slice(0, NC_), ln,
                    [xTo_sb[:, ko, :] for ko in range(KO)], False, None, NC_)
            return ln

        def wj_tile(w_d, j):
            """stream one 128-col slice of a [E, E] fp8 weight into SBUF."""
            w_sb = pb2_ref[0].tile([128, KO, 128], BF16, tag="wj", bufs=4, name="wj")
            nc.sync.dma_start(
                out=w_sb[:],
                in_=w_d.rearrange("(ko p) m -> p ko m", p=128)[:, :, j * 128:(j + 1) * 128])
            return w_sb

        def project_qt(lnsrc, lncols, w_d):
            """Q^T for 16 heads over own tokens -> [128, 8, 512]."""
            qt = pb2_ref[0].tile([128, 8, NC_], BF16, tag="qon", bufs=2, name="qt")
            for j in range(8):
                w_sb = wj_tile(w_d, j)
                p_ = ps()
                for ko in range(KO):
                    nc.tensor.matmul(p_[:, :], w_sb[:, ko, :],
                                     lnsrc[:, ko, lncols], start=(ko == 0),
                                     stop=(ko == KO - 1))
                nc.vector.tensor_copy(qt[:, j, :], p_[:, :])
            return qt

        def project_kt(src, w_d):
            """K^T for 16 heads over full S -> [128, 8, 1024]."""
            kt = pb2_ref[0].tile([128, 8, T], BF16, tag="kt", name="kt")
            for j in range(8):
                w_sb = wj_tile(w_d, j)
                for c in range(CC):
                    p_ = ps()
                    for ko in range(KO):
                        nc.tensor.matmul(p_[:, :], w_sb[:, ko, :],
                                         src[:, ko, c * NC_:(c + 1) * NC_],
                                         start=(ko == 0), stop=(ko == KO - 1))
                    nc.vector.tensor_copy(kt[:, j, c * NC_:(c + 1) * NC_], p_[:, :])
            return kt

        def project_v(src, w_d, flag_other=False):
            """V for 16 heads -> [128, 8, 16, 65].  When flag_other is set,
            key blocks 4..7 (the pair's other T-half) are scaled by the
            per-core visibility flag (0 for the early-half core, 1 for the
            late-half core) so self-attention needs no mask there."""
            vv = pb2_ref[0].tile([128, 8, H, 65], BF16, tag="vv", name="vv")
            for q4 in range(4):
                w_sb = pb2_ref[0].tile([128, KO, 256], BF16, tag="wv4", bufs=2, name="wsb")
                nc.sync.dma_start(
                    out=w_sb[:],
                    in_=w_d.rearrange("(ko p) m -> p ko m", p=128)[:, :, q4 * 256:(q4 + 1) * 256])
                hs = slice(q4 * 4, (q4 + 1) * 4)
                for s in range(8):
                    p_ = ps()
                    for ko in range(KO):
                        nc.tensor.matmul(p_[:, 0:256], src[:, ko, s * 128:(s + 1) * 128],
                                         w_sb[:, ko, :], start=(ko == 0),
                                         stop=(ko == KO - 1))
                    if flag_other and s >= 4:
                        nc.scalar.activation(
                            vv[:, s, hs, 0:64],
                            p_[:, 0:256].rearrange("p (h d) -> p h d", d=64),
                            AF.Copy, scale=fl_sb[:, 0:1])
                        nc.scalar.activation(
                            vv[:, s, hs, 64:65].rearrange("p h o -> p (h o)"),
                            ones_bf[:, 0:4], AF.Copy, scale=fl_sb[:, 0:1])
                    else:
                        nc.scalar.copy(out=vv[:, s, hs, 0:64],
                                       in_=p_[:, 0:256].rearrange("p (h d) -> p h d", d=64))
                        nc.vector.memset(vv[:, s, hs, 64:65], 1.0)
            return vv

        def attention(qt, kt, vv, onorm, masked):
            for h_ in range(H):
                j, half = h_ // 2, h_ % 2
                pb = 64 * half
                eb = pb2_ref[0].tile([128, 8, NC_], BF16, tag="expb", bufs=2, name="eb")
                for s2 in range(4):          # pairs of key blocks share one
                    pw = psw()               # 2-bank PSUM tile + one wide exp
                    for q_ in range(2):
                        s_ = 2 * s2 + q_
                        nc.tensor.matmul(pw[:, q_ * NC_:(q_ + 1) * NC_],
                                         kt[pb:pb + 64, j, s_ * 128:(s_ + 1) * 128],
                                         qt[pb:pb + 64, j, :], start=True, stop=True)
                    nc.scalar.activation(
                        eb[:, 2 * s2:2 * s2 + 2, :].rearrange("p a b -> p (a b)"),
                        pw[:, :], AF.Exp, scale=float(E) ** -0.5)
                    if masked and s2 < 2:
                        nc.vector.tensor_mul(
                            eb[:, 2 * s2:2 * s2 + 2, :].rearrange("p a b -> p (a b)"),
                            eb[:, 2 * s2:2 * s2 + 2, :].rearrange("p a b -> p (a b)"),
                            sm_sb[:, 2 * s2:2 * s2 + 2, :].rearrange("p a b -> p (a b)"))
                av = ps()
                for s_ in range(8):
                    nc.tensor.matmul(av[0:65, :], vv[:, s_, h_, :], eb[:, s_, :],
                                     start=(s_ == 0), stop=(s_ == 7))
                rb = pb2_ref[0].tile([65, NC_], BF16, tag="row_rrb", bufs=2, name="rb")
                with nc.allow_low_precision(reason="1/denominator rounds to bf16 anyway"):
                    nc.vector.reciprocal(rb[64:65, :], av[64:65, :])
                bc = ps()
                nc.tensor.matmul(bc[0:64, :], ones_bf[64:65, 0:64], rb[64:65, :],
                                 start=True, stop=True)
                bcs = pb2_ref[0].tile([64, NC_], BF16, tag="bcs", bufs=2, name="bcs")
                nc.vector.tensor_copy(bcs[:, :], bc[0:64, :])
                nc.vector.tensor_tensor(onorm[pb:pb + 64, j, :],
                                        av[0:64, :], bcs[:, :], OP.mult)

        def out_proj(onorm, wo_d, bias_pp, ln_stats=None):
            """project + residual-add; optionally interleave next-LN stats so
            they overlap the projection instead of stalling PE afterwards."""
            for m in range(KO):
                wom = pb2_ref[0].tile([128, KO, 128], BF16, tag="wom", bufs=2, name="wom")
                nc.sync.dma_start(
                    out=wom[:],
                    in_=wo_d.rearrange("(ks p) e -> p ks e", p=128)[:, :, m * 128:(m + 1) * 128])
                p_ = ps()
                for ks in range(KO):
                    nc.tensor.matmul(p_[:, :], wom[:, ks, :], onorm[:, ks, :],
                                     start=(ks == 0), stop=(ks == KO - 1))
                nc.vector.scalar_tensor_tensor(xTo_sb[:, m, :], p_[:, :],
                                               bias_pp[:, m:m + 1], xTo_sb[:, m, :],
                                               OP.add, OP.add)
                if ln_stats is not None:
                    ln_stats_step(ln_stats, m)

        def ln_stats_tiles():
            xb = pa.tile([128, KO, NC_], BF16, tag="stat", bufs=2, name="xb")
            sq = pa.tile([128, KO, NC_], BF16, tag="stat", bufs=2, name="sq")
            return (xb, sq, ps(), ps())

        with tc.tile_pool(name=f"B{ibody}", bufs=1) as _pb2:
            pb2_ref[0] = _pb2
            # ---- self attention ----
            ln1 = layer_norm1()
            qt = project_qt(ln1, slice(0, NC_), W["wq"])
            kt = project_kt(ln1, W["wk"])
            vv = project_v(ln1, W["wv"], flag_other=True)
            on1 = _pb2.tile([128, 8, NC_], BF16, tag="qon", bufs=2, name="on1")
            attention(qt, kt, vv, on1, masked=True)
            st2 = ln_stats_tiles()
            out_proj(on1, W["wo"], bopp, ln_stats=st2)

            # ---- cross attention ----
            # cross K from raw ca is independent of LN2 - it keeps PE busy
            # while the serial LN2 row chain runs on DVE/ACT
            ktc = project_kt(ca_sb, W["wkc"])
            ln2 = layer_norm_h_rows(1, st2)
            qtc = project_qt(ln2, slice(0, NC_), W["wqc"])
            vvc = project_v(ca_sb, W["wvc"])
            on2 = _pb2.tile([128, 8, NC_], BF16, tag="qon", bufs=2, name="on2")
            attention(qtc, ktc, vvc, on2, masked=False)
            st3 = ln_stats_tiles()
            out_proj(on2, W["woc"], bocpp, ln_stats=st3)

        # ---- FFN ----
        ln3 = layer_norm_h_rows(2, st3)
        with tc.tile_pool(name=f"C{ibody}", bufs=1) as pc:
            FH = 4 * E // 128
            ht = pc.tile([128, FH, NC_], BF16, tag="ht", name="ht")
            for m in range(FH):
                w1m = pc.tile([128, KO, 128], BF16, tag="w1m", bufs=4, name="w1m")
                nc.sync.dma_start(
                    out=w1m[:],
                    in_=W["w1"].rearrange("(ko p) f -> p ko f", p=128)[:, :, m * 128:(m + 1) * 128])
                p_ = ps()
                for ko in range(KO):
                    nc.tensor.matmul(p_[:, :], w1m[:, ko, :], ln3[:, ko, :],
                                     start=(ko == 0), stop=(ko == KO - 1))
                nc.scalar.activation(ht[:, m, :], p_[:, :], AF.Relu,
                                     bias=b1pp[:, m:m + 1])
            for m in range(KO):
                w2m = pc.tile([128, FH, 128], BF16, tag="w2m", bufs=2, name="w2m")
                nc.sync.dma_start(
                    out=w2m[:],
                    in_=W["w2"].rearrange("(ks p) e -> p ks e", p=128)[:, :, m * 128:(m + 1) * 128])
                p_ = ps()
                for ks in range(FH):
                    nc.tensor.matmul(p_[:, :], w2m[:, ks, :], ht[:, ks, :],
                                     start=(ks == 0), stop=(ks == FH - 1))
                nc.vector.scalar_tensor_tensor(xTo_sb[:, m, :], p_[:, :],
                                               b2pp[:, m:m + 1], xTo_sb[:, m, :],
                                               OP.add, OP.add)
        for ko in range(KO):
            nc.sync.dma_start(out=out_xT[ko * 128:(ko + 1) * 128, :],
                              in_=xTo_sb[:, ko, :])


def _make_core_inputs_nocc(c, inp):
    bf = ml_dtypes.bfloat16
    f8 = ml_dtypes.float8_e4m3
    b, h = divmod(c, 2)
    own = slice(512 * h, 512 * h + 512)
    oth = slice(512 * (1 - h), 512 * (1 - h) + 512)

    def stack_heads(w):  # [16, E, D] -> [E, 1024]
        return np.ascontiguousarray(np.transpose(w, (1, 0, 2)).reshape(E, E)).astype(bf)

    xt = np.asarray(inp["x"][b], np.float32)           # [T, E]
    xperm = np.concatenate([xt[own], xt[oth]], axis=0)  # keys permuted: own first
    # own-half causal mask (key blocks 0..3 in permuted order): sp <= f
    sp, f = np.arange(512), np.arange(512)
    mask = (sp[:, None] <= f[None, :])                  # [512, 512]
    smask = mask.reshape(4, 128, 512).transpose(1, 0, 2).astype(bf)

    return {
        "xTb": np.ascontiguousarray(xperm.T).astype(bf),
        "xTo": np.ascontiguousarray(xt[own].T).astype(np.float32),
        "caT": np.ascontiguousarray(np.asarray(inp["ca"][b]).T).astype(bf),
        "wq": stack_heads(inp["Wq_s"]),
        "wk": stack_heads(inp["Wk_s"]),
        "wv": stack_heads(inp["Wv_s"]),
        "wqc": stack_heads(inp["Wq_c"]),
        "wkc": stack_heads(inp["Wk_c"]),
        "wvc": stack_heads(inp["Wv_c"]),
        "wo": np.asarray(inp["Wo_s"], np.float32).astype(bf),
        "woc": np.asarray(inp["Wo_c"], np.float32).astype(bf),
        "w1": np.asarray(inp["W1"], np.float32).astype(bf),
        "w2": np.asarray(inp["W2"], np.float32).astype(bf),
        "g1": np.asarray(inp["ln1_g"], np.float32),
        "g2": np.asarray(inp["ln2_g"], np.float32),
        "g3": np.asarray(inp["ln3_g"], np.float32),
        "bl1": np.asarray(inp["ln1_b"], np.float32),
        "bl2": np.asarray(inp["ln2_b"], np.float32),
        "bl3": np.asarray(inp["ln3_b"], np.float32),
        "bo": np.asarray(inp["bo_s"], np.float32),
        "boc": np.asarray(inp["bo_c"], np.float32),
        "b2": np.asarray(inp["b2"], np.float32),
        "b1r": np.asarray(inp["b1"], np.float32),
        "smask": smask,
        "oflag": np.full((128, 1), float(h), np.float32),
    }

